# revision 1
# baseline (speedup 1.0000x reference)
"""DistMult edge scoring on 8 Trainium2 NeuronCores.

score[e] = sum_d node_emb[src[e], d] * rel_emb[e, d] * node_emb[dst[e], d]

Strategy (data-parallel over edges, per the sharding hint):
  - Edges sharded contiguously across 8 cores (125k/core, padded to whole
    128x32 tiles). Node table replicated per core in HBM.
  - Gather: gpsimd dma_gather is descriptor-count-bound (~8 ns/descriptor,
    measured), descriptor SIZE is nearly free, and int16 indices cap the
    addressable range at 32767. So the table is viewed as 25000 groups of
    4 rows (1 KB each): one descriptor fetches an edge's whole 4-row group
    (group id = node >> 2 < 25000 fits int16), and DVE selects the right
    row with host-precomputed 0/1 masks (node & 3). One descriptor per
    edge per direction - 4x fewer than any windowed row-gather scheme.
  - Per tile of 4096 edges: 2 gathers (one per direction, 4096 indices
    each in multi-packet mode), one rel load, DVE mask-select + product +
    segmented reduce, scores out. Measured: ~7.9 ns/descriptor, GpSimd
    descgen-saturated (~50 us idle in 2.1 ms).
  - Raw bacc with a manually pipelined 2-slot schedule (Tile does not
    semaphorize InstDMAGatherAnt).
"""

import numpy as np

import concourse.bacc as bacc
import concourse.bass as bass
import concourse.mybir as mybir
from concourse import library_config
from concourse.bass_utils import run_bass_kernel_spmd

N_NODES = 100000
DIM = 64
N_EDGES = 1000000
N_CORES = 8

P = 128
K = 32
TILE = P * K                      # 4096 edges per tile
EPC = N_EDGES // N_CORES          # 125000
NT = -(-EPC // TILE)              # 31
EPAD = NT * TILE                  # 126976

GR = 4                            # rows per gather group
NGRP = N_NODES // GR              # 25000 groups, fits int16
CHUNK = TILE                      # idx per dma_gather (multi-packet mode)
CPT = TILE // CHUNK               # 1 chunk per tile
QC = CHUNK // P                   # 32 free slots per chunk

IDXF = TILE // 16                 # 256 int16s per partition per direction
F32 = mybir.dt.float32

_cache = {}


def _chunk_valid(t, c):
    """Valid (non-pad) edge count in tile t, chunk c; pad idx are trailing
    -1s skipped by num_idxs_reg semantics. All-pad chunks keep one valid
    index (host forces slot 0 to group 0) so the DMA completion fires."""
    lo = t * TILE + c * CHUNK
    return int(max(1, min(CHUNK, EPC - lo)))


def _build_program():
    if "nc" in _cache:
        return _cache["nc"]

    nc = bacc.Bacc(
        "TRN2",
        target_bir_lowering=False,
        debug=False,
        enable_asserts=False,
        num_devices=N_CORES,
    )
    table = nc.dram_tensor("table", [NGRP, GR * DIM], F32, kind="ExternalInput")
    idx_h = nc.dram_tensor(
        "idx", [NT, P, 2, IDXF], mybir.dt.int16, kind="ExternalInput"
    )
    rel_h = nc.dram_tensor("rel", [NT, P, K, DIM], F32, kind="ExternalInput")
    msk_h = nc.dram_tensor("msk", [NT, P, 2, GR, K], F32, kind="ExternalInput")
    out_h = nc.dram_tensor("score", [NT, P, K], F32, kind="ExternalOutput")

    NB = 2     # gather-buffer pipeline slots
    NB_IO = 4  # idx/rel/msk prefetch slots
    NOPS = 17  # DVE ops per tile

    with (
        nc.Block() as block,
        nc.sbuf_tensor("gbuf", [P, NB, 2, CPT, QC, GR * DIM], F32) as gbuf,
        nc.sbuf_tensor("relb", [P, NB_IO, K, DIM], F32) as relb,
        nc.sbuf_tensor("mskb", [P, NB_IO, 2, GR, K], F32) as mskb,
        nc.sbuf_tensor("idxb", [P, NB_IO, 2, IDXF], mybir.dt.int16) as idxb,
        nc.sbuf_tensor("hbuf", [P, K, DIM], F32) as hbuf,
        nc.sbuf_tensor("tbuf", [P, K, DIM], F32) as tbuf,
        nc.sbuf_tensor("tmpb", [P, K, DIM], F32) as tmpb,
        nc.sbuf_tensor("sbuf_s", [P, NB, K], F32) as sb_s,
        nc.semaphore("s_idx") as s_idx,
        nc.semaphore("s_rel") as s_rel,
        nc.semaphore("s_msk") as s_msk,
        nc.semaphore("s_g") as s_g,
        nc.semaphore("s_g2") as s_g2,
        nc.semaphore("s_g3") as s_g3,
        nc.semaphore("s_g4") as s_g4,
        nc.semaphore("s_vc") as s_vc,
        nc.semaphore("s_out") as s_out,
    ):

        @block.sync
        def _(sp: bass.BassEngine):
            # pure prefetcher: never gated on the compute chain beyond
            # slot reuse (NB_IO slots deep)
            for t in range(NT):
                s = t % NB_IO
                if t >= 1:
                    # order completions: sem count N must imply tiles 0..N-1
                    # are actually resident (DMAs can finish out of order)
                    sp.wait_ge(s_idx, 16 * t)
                    sp.wait_ge(s_rel, 16 * t)
                    sp.wait_ge(s_msk, 16 * t)
                if t >= NB_IO:
                    # idx slot free once tile t-NB_IO's gathers retired
                    tt = t - NB_IO
                    sp.wait_ge((s_g, s_g2)[tt % 2], 16 * (tt // 2 + 1))
                    sp.wait_ge((s_g3, s_g4)[tt % 2], 16 * (tt // 2 + 1))
                sp.dma_start(out=idxb[:, s], in_=idx_h[t]).then_inc(s_idx, 16)
                if t >= NB_IO:
                    # rel/msk slots consumed by DVE of tile t-NB_IO
                    sp.wait_ge(s_vc, NOPS * (t - NB_IO + 1))
                sp.dma_start(out=relb[:, s], in_=rel_h[t]).then_inc(s_rel, 16)
                sp.dma_start(out=mskb[:, s], in_=msk_h[t]).then_inc(s_msk, 16)

        @block.scalar
        def _(sc: bass.BassEngine):
            # out-stores, decoupled from the prefetch stream
            for t in range(NT):
                sc.wait_ge(s_vc, NOPS * (t + 1))
                if t >= 1:
                    sc.wait_ge(s_out, 16 * t)
                sc.dma_start(
                    out=out_h[t], in_=sb_s[:, t % NB]
                ).then_inc(s_out, 16)
            sc.wait_ge(s_out, 16 * NT)

        @block.gpsimd
        def _(gp: bass.BassGpSimd):
            gp.load_library(library_config.mlp)
            for t in range(NT):
                s = t % NB
                gp.wait_ge(s_idx, 16 * (t + 1))
                if t >= NB:
                    # gather buffers of tile t-NB consumed by DVE
                    gp.wait_ge(s_vc, NOPS * (t - NB + 1))
                for d in range(2):
                    gp.dma_gather(
                        gbuf[:, s, d, 0],
                        table[:],
                        idxb[:, t % NB_IO, d],
                        CHUNK,
                        _chunk_valid(t, 0),
                        GR * DIM,
                        elem_step=GR * DIM,
                        single_packet=False,
                    ).then_inc(
                        (s_g, s_g2)[t % 2] if d == 0 else (s_g3, s_g4)[t % 2],
                        16,
                    )

        @block.vector
        def _(v: bass.BassEngine):
            mult = mybir.AluOpType.mult
            add = mybir.AluOpType.add
            for t in range(NT):
                s = t % NB
                v.wait_ge((s_g, s_g2)[t % 2], 16 * (t // 2 + 1))  # src landed
                v.wait_ge(s_rel, 16 * (t + 1))
                v.wait_ge(s_msk, 16 * (t + 1))
                if t >= NB:
                    v.wait_ge(s_out, 16 * (t - NB + 1))
                if t >= 1:
                    # hbuf/tbuf/tmpb WAR vs previous tile's chain
                    v.wait_ge(s_vc, NOPS * t)
                # last tile: only ceil(valid/P) k-slots hold real edges
                KV = K if t < NT - 1 else -(-(EPC - t * TILE) // P)
                i = NOPS * t

                def op(instr):
                    nonlocal i
                    i += 1
                    instr.then_inc(s_vc, 1)

                def wait():
                    v.wait_ge(s_vc, i)

                for d, dst in ((0, hbuf), (1, tbuf)):
                    if d == 1:
                        v.wait_ge((s_g3, s_g4)[t % 2], 16 * (t // 2 + 1))
                    # g viewed [P, K, GR*DIM]: (c, q) -> k = c*QC + q
                    g = gbuf[:, s, d].rearrange("p c q e -> p (c q) e")
                    for r in range(GR):
                        m = mskb[:, t % NB_IO, d, r, :KV].to_broadcast(
                            [P, KV, DIM]
                        )
                        gsl = g[:, :KV, r * DIM : (r + 1) * DIM]
                        if r == 0:
                            op(
                                v.tensor_tensor(
                                    out=dst[:, :KV], in0=gsl, in1=m, op=mult
                                )
                            )
                        else:
                            wait()
                            op(
                                v.tensor_tensor(
                                    out=tmpb[:, :KV], in0=gsl, in1=m, op=mult
                                )
                            )
                            wait()
                            op(
                                v.tensor_tensor(
                                    out=dst[:, :KV],
                                    in0=dst[:, :KV],
                                    in1=tmpb[:, :KV],
                                    op=add,
                                )
                            )
                wait()
                op(
                    v.tensor_tensor(
                        out=hbuf[:, :KV],
                        in0=hbuf[:, :KV],
                        in1=relb[:, t % NB_IO, :KV],
                        op=mult,
                    )
                )
                wait()
                op(
                    v.tensor_tensor(
                        out=hbuf[:, :KV], in0=hbuf[:, :KV], in1=tbuf[:, :KV],
                        op=mult,
                    )
                )
                wait()
                v.tensor_reduce(
                    out=sb_s[:, s, :KV],
                    in_=hbuf[:, :KV],
                    axis=mybir.AxisListType.X,
                    op=add,
                ).then_inc(s_vc, 1)

    nc.compile()
    _cache["nc"] = nc
    return nc


def _prep_idx(idx_global):
    """(EPAD,) node ids -> wrapped int16 group indices [NT, P, IDXF].
    Pad edges (index < 0) stay -1 and are skipped via num_idxs_reg."""
    n = idx_global.reshape(NT, TILE)
    g = np.where(n >= 0, n >> 2, -1).astype(np.int16)
    for t in range(NT):
        for c in range(CPT):
            if t * TILE + c * CHUNK >= EPC:
                g[t, c * CHUNK] = 0  # keep >=1 valid idx per chunk
    # wrap: index j -> [j % 16, j // 16], replicated across 8 partition groups
    wr = g.reshape(NT, IDXF, 16).swapaxes(1, 2)  # [NT, 16, IDXF]
    return np.broadcast_to(wr[:, None, :, :], (NT, 8, 16, IDXF)).reshape(
        NT, P, IDXF
    )


def _prep_msk(idx_global):
    """(EPAD,) node ids -> 0/1 row-select masks [NT, P, GR, K]."""
    sub = (np.maximum(idx_global.reshape(NT, K, P), 0) & 3).astype(np.int8)
    m = (sub[:, None, :, :] == np.arange(GR, dtype=np.int8)[None, :, None, None])
    # [NT, GR, K, P] -> [NT, P, GR, K]
    return np.ascontiguousarray(m.transpose(0, 3, 1, 2)).astype(np.float32)


def _shard_inputs(node_emb, rel_emb, src, dst):
    node_emb = np.asarray(node_emb, dtype=np.float32)
    rel_emb = np.asarray(rel_emb, dtype=np.float32)
    src = np.asarray(src, dtype=np.int64)
    dst = np.asarray(dst, dtype=np.int64)

    table = np.ascontiguousarray(node_emb.reshape(NGRP, GR * DIM))

    in_maps = []
    for c in range(N_CORES):
        sl = slice(c * EPC, (c + 1) * EPC)
        src_c = np.full(EPAD, -1, np.int64)
        dst_c = np.full(EPAD, -1, np.int64)
        rel_c = np.zeros((EPAD, DIM), np.float32)
        src_c[:EPC] = src[sl]
        dst_c[:EPC] = dst[sl]
        rel_c[:EPC] = rel_emb[sl]

        idx = np.stack([_prep_idx(src_c), _prep_idx(dst_c)], axis=2)
        # [NT, P, 2, IDXF]
        msk = np.stack([_prep_msk(src_c), _prep_msk(dst_c)], axis=2)
        # [NT, P, 2, GR, K]
        # edge j at [p = j % 128, k = j // 128] -> rel[t, p, k]
        rel_t = np.ascontiguousarray(
            rel_c.reshape(NT, K, P, DIM).swapaxes(1, 2)
        )
        in_maps.append(
            {
                "table": table,
                "idx": np.ascontiguousarray(idx),
                "rel": rel_t,
                "msk": msk,
            }
        )
    return in_maps


def run_on_hw(node_emb, rel_emb, src, dst, **spmd_kwargs):
    nc = _build_program()
    in_maps = _shard_inputs(node_emb, rel_emb, src, dst)
    res = run_bass_kernel_spmd(nc, in_maps, list(range(N_CORES)), **spmd_kwargs)
    parts = [
        np.asarray(res.results[c]["score"])
        .transpose(0, 2, 1)
        .reshape(EPAD)[:EPC]
        for c in range(N_CORES)
    ]
    return np.concatenate(parts), res


def kernel(node_emb, rel_emb, src, dst):
    scores, _ = run_on_hw(node_emb, rel_emb, src, dst)
    return scores



# revision 3
# speedup vs baseline: 1.4833x; 1.4833x over previous
"""DistMult edge scoring on 8 Trainium2 NeuronCores.

score[e] = sum_d node_emb[src[e], d] * rel_emb[e, d] * node_emb[dst[e], d]

Strategy (data-parallel over edges, per the sharding hint):
  - Edges sharded contiguously across 8 cores (125k/core, padded to whole
    128x32 tiles). Node table replicated per core in HBM.
  - Gather: gpsimd dma_gather is descriptor-count-bound (~8 ns/descriptor,
    measured), descriptor SIZE is nearly free, and int16 indices cap the
    addressable range at 32767. So the table is viewed as 25000 groups of
    4 rows (1 KB each): one descriptor fetches an edge's whole 4-row group
    (group id = node >> 2 < 25000 fits int16), and DVE selects the right
    row with host-precomputed 0/1 masks (node & 3). One descriptor per
    edge per direction - 4x fewer than any windowed row-gather scheme.
  - Per tile of 4096 edges: 2 gathers (one per direction, 4096 indices
    each in multi-packet mode), one rel load, DVE mask-select + product +
    segmented reduce, scores out. Measured: ~7.9 ns/descriptor, GpSimd
    descgen-saturated (~50 us idle in 2.1 ms).
  - Raw bacc with a manually pipelined 2-slot schedule (Tile does not
    semaphorize InstDMAGatherAnt).
"""

import numpy as np

import concourse.bacc as bacc
import concourse.bass as bass
import concourse.mybir as mybir
from concourse import library_config
from concourse.bass_utils import run_bass_kernel_spmd

N_NODES = 100000
DIM = 64
N_EDGES = 1000000
N_CORES = 8

P = 128
K = 32
TILE = P * K                      # 4096 edges per tile
EPC = N_EDGES // N_CORES          # 125000
NT = -(-EPC // TILE)              # 31
EPAD = NT * TILE                  # 126976

GR = 4                            # rows per gather group
NGRP = N_NODES // GR              # 25000 groups, fits int16
CHUNK = TILE                      # idx per dma_gather (multi-packet mode)
CPT = TILE // CHUNK               # 1 chunk per tile
QC = CHUNK // P                   # 32 free slots per chunk

IDXF = TILE // 16                 # 256 int16s per partition per direction
F32 = mybir.dt.float32

_cache = {}


def _chunk_valid(t, c):
    """Valid (non-pad) edge count in tile t, chunk c; pad idx are trailing
    -1s skipped by num_idxs_reg semantics. All-pad chunks keep one valid
    index (host forces slot 0 to group 0) so the DMA completion fires."""
    lo = t * TILE + c * CHUNK
    return int(max(1, min(CHUNK, EPC - lo)))


def _build_program():
    if "nc" in _cache:
        return _cache["nc"]

    nc = bacc.Bacc(
        "TRN2",
        target_bir_lowering=False,
        debug=False,
        enable_asserts=False,
        num_devices=N_CORES,
        num_swdge_queues=4,
    )
    table = nc.dram_tensor("table", [NGRP, GR * DIM], F32, kind="ExternalInput")
    idx_h = nc.dram_tensor(
        "idx", [NT, P, 2, IDXF], mybir.dt.int16, kind="ExternalInput"
    )
    rel_h = nc.dram_tensor("rel", [NT, P, K, DIM], F32, kind="ExternalInput")
    msk_h = nc.dram_tensor("msk", [NT, P, 2, GR, K], F32, kind="ExternalInput")
    out_h = nc.dram_tensor("score", [NT, P, K], F32, kind="ExternalOutput")

    NB = 2     # gather-buffer pipeline slots
    NB_IO = 4  # idx/rel/msk prefetch slots
    NOPS = 17  # DVE ops per tile

    with (
        nc.Block() as block,
        nc.sbuf_tensor("gbuf", [P, NB, 2, CPT, QC, GR * DIM], F32) as gbuf,
        nc.sbuf_tensor("relb", [P, NB_IO, K, DIM], F32) as relb,
        nc.sbuf_tensor("mskb", [P, NB_IO, 2, GR, K], F32) as mskb,
        nc.sbuf_tensor("idxb", [P, NB_IO, 2, IDXF], mybir.dt.int16) as idxb,
        nc.sbuf_tensor("hbuf", [P, K, DIM], F32) as hbuf,
        nc.sbuf_tensor("tbuf", [P, K, DIM], F32) as tbuf,
        nc.sbuf_tensor("tmpb", [P, K, DIM], F32) as tmpb,
        nc.sbuf_tensor("sbuf_s", [P, NB, K], F32) as sb_s,
        nc.semaphore("s_idx") as s_idx,
        nc.semaphore("s_rel") as s_rel,
        nc.semaphore("s_msk") as s_msk,
        nc.semaphore("s_g") as s_g,
        nc.semaphore("s_g2") as s_g2,
        nc.semaphore("s_g3") as s_g3,
        nc.semaphore("s_g4") as s_g4,
        nc.semaphore("s_vc") as s_vc,
        nc.semaphore("s_out") as s_out,
    ):

        @block.sync
        def _(sp: bass.BassEngine):
            # pure prefetcher: never gated on the compute chain beyond
            # slot reuse (NB_IO slots deep)
            for t in range(NT):
                s = t % NB_IO
                if t >= 1:
                    # order completions: sem count N must imply tiles 0..N-1
                    # are actually resident (DMAs can finish out of order)
                    sp.wait_ge(s_idx, 16 * t)
                    sp.wait_ge(s_rel, 16 * t)
                    sp.wait_ge(s_msk, 16 * t)
                if t >= NB_IO:
                    # idx slot free once tile t-NB_IO's gathers retired
                    tt = t - NB_IO
                    sp.wait_ge((s_g, s_g2)[tt % 2], 16 * (tt // 2 + 1))
                    sp.wait_ge((s_g3, s_g4)[tt % 2], 16 * (tt // 2 + 1))
                sp.dma_start(out=idxb[:, s], in_=idx_h[t]).then_inc(s_idx, 16)
                if t >= NB_IO:
                    # rel/msk slots consumed by DVE of tile t-NB_IO
                    sp.wait_ge(s_vc, NOPS * (t - NB_IO + 1))
                sp.dma_start(out=relb[:, s], in_=rel_h[t]).then_inc(s_rel, 16)
                sp.dma_start(out=mskb[:, s], in_=msk_h[t]).then_inc(s_msk, 16)

        @block.scalar
        def _(sc: bass.BassEngine):
            # out-stores, decoupled from the prefetch stream
            for t in range(NT):
                sc.wait_ge(s_vc, NOPS * (t + 1))
                if t >= 1:
                    sc.wait_ge(s_out, 16 * t)
                sc.dma_start(
                    out=out_h[t], in_=sb_s[:, t % NB]
                ).then_inc(s_out, 16)
            sc.wait_ge(s_out, 16 * NT)

        @block.gpsimd
        def _(gp: bass.BassGpSimd):
            gp.load_library(library_config.mlp)
            for t in range(NT):
                s = t % NB
                gp.wait_ge(s_idx, 16 * (t + 1))
                if t >= NB:
                    # gather buffers of tile t-NB consumed by DVE
                    gp.wait_ge(s_vc, NOPS * (t - NB + 1))
                # queue = 2*(t%2)+d: each (t%2, d) sem-stream stays on one
                # queue (FIFO completion order within a queue keeps the sem
                # count monotone with tile index). d=1 issued first: queue 0
                # blocks the engine for the whole descgen; 1-3 return fast
                # and generate in the background on their own Q7 pairs.
                for d in (1, 0):
                    gp.dma_gather(
                        gbuf[:, s, d, 0],
                        table[:],
                        idxb[:, t % NB_IO, d],
                        CHUNK,
                        _chunk_valid(t, 0),
                        GR * DIM,
                        elem_step=GR * DIM,
                        single_packet=False,
                        queue_num=2 * (t % 2) + d,
                    ).then_inc(
                        (s_g, s_g2)[t % 2] if d == 0 else (s_g3, s_g4)[t % 2],
                        16,
                    )

        @block.vector
        def _(v: bass.BassEngine):
            mult = mybir.AluOpType.mult
            add = mybir.AluOpType.add
            for t in range(NT):
                s = t % NB
                v.wait_ge((s_g, s_g2)[t % 2], 16 * (t // 2 + 1))  # src landed
                v.wait_ge(s_rel, 16 * (t + 1))
                v.wait_ge(s_msk, 16 * (t + 1))
                if t >= NB:
                    v.wait_ge(s_out, 16 * (t - NB + 1))
                if t >= 1:
                    # hbuf/tbuf/tmpb WAR vs previous tile's chain
                    v.wait_ge(s_vc, NOPS * t)
                # last tile: only ceil(valid/P) k-slots hold real edges
                KV = K if t < NT - 1 else -(-(EPC - t * TILE) // P)
                i = NOPS * t

                def op(instr):
                    nonlocal i
                    i += 1
                    instr.then_inc(s_vc, 1)

                def wait():
                    v.wait_ge(s_vc, i)

                for d, dst in ((0, hbuf), (1, tbuf)):
                    if d == 1:
                        v.wait_ge((s_g3, s_g4)[t % 2], 16 * (t // 2 + 1))
                    # g viewed [P, K, GR*DIM]: (c, q) -> k = c*QC + q
                    g = gbuf[:, s, d].rearrange("p c q e -> p (c q) e")
                    for r in range(GR):
                        m = mskb[:, t % NB_IO, d, r, :KV].to_broadcast(
                            [P, KV, DIM]
                        )
                        gsl = g[:, :KV, r * DIM : (r + 1) * DIM]
                        if r == 0:
                            op(
                                v.tensor_tensor(
                                    out=dst[:, :KV], in0=gsl, in1=m, op=mult
                                )
                            )
                        else:
                            wait()
                            op(
                                v.tensor_tensor(
                                    out=tmpb[:, :KV], in0=gsl, in1=m, op=mult
                                )
                            )
                            wait()
                            op(
                                v.tensor_tensor(
                                    out=dst[:, :KV],
                                    in0=dst[:, :KV],
                                    in1=tmpb[:, :KV],
                                    op=add,
                                )
                            )
                wait()
                op(
                    v.tensor_tensor(
                        out=hbuf[:, :KV],
                        in0=hbuf[:, :KV],
                        in1=relb[:, t % NB_IO, :KV],
                        op=mult,
                    )
                )
                wait()
                op(
                    v.tensor_tensor(
                        out=hbuf[:, :KV], in0=hbuf[:, :KV], in1=tbuf[:, :KV],
                        op=mult,
                    )
                )
                wait()
                v.tensor_reduce(
                    out=sb_s[:, s, :KV],
                    in_=hbuf[:, :KV],
                    axis=mybir.AxisListType.X,
                    op=add,
                ).then_inc(s_vc, 1)

    nc.compile()
    _cache["nc"] = nc
    return nc


def _prep_idx(idx_global):
    """(EPAD,) node ids -> wrapped int16 group indices [NT, P, IDXF].
    Pad edges (index < 0) stay -1 and are skipped via num_idxs_reg."""
    n = idx_global.reshape(NT, TILE)
    g = np.where(n >= 0, n >> 2, -1).astype(np.int16)
    for t in range(NT):
        for c in range(CPT):
            if t * TILE + c * CHUNK >= EPC:
                g[t, c * CHUNK] = 0  # keep >=1 valid idx per chunk
    # wrap: index j -> [j % 16, j // 16], replicated across 8 partition groups
    wr = g.reshape(NT, IDXF, 16).swapaxes(1, 2)  # [NT, 16, IDXF]
    return np.broadcast_to(wr[:, None, :, :], (NT, 8, 16, IDXF)).reshape(
        NT, P, IDXF
    )


def _prep_msk(idx_global):
    """(EPAD,) node ids -> 0/1 row-select masks [NT, P, GR, K]."""
    sub = (np.maximum(idx_global.reshape(NT, K, P), 0) & 3).astype(np.int8)
    m = (sub[:, None, :, :] == np.arange(GR, dtype=np.int8)[None, :, None, None])
    # [NT, GR, K, P] -> [NT, P, GR, K]
    return np.ascontiguousarray(m.transpose(0, 3, 1, 2)).astype(np.float32)


def _shard_inputs(node_emb, rel_emb, src, dst):
    node_emb = np.asarray(node_emb, dtype=np.float32)
    rel_emb = np.asarray(rel_emb, dtype=np.float32)
    src = np.asarray(src, dtype=np.int64)
    dst = np.asarray(dst, dtype=np.int64)

    table = np.ascontiguousarray(node_emb.reshape(NGRP, GR * DIM))

    in_maps = []
    for c in range(N_CORES):
        sl = slice(c * EPC, (c + 1) * EPC)
        src_c = np.full(EPAD, -1, np.int64)
        dst_c = np.full(EPAD, -1, np.int64)
        rel_c = np.zeros((EPAD, DIM), np.float32)
        src_c[:EPC] = src[sl]
        dst_c[:EPC] = dst[sl]
        rel_c[:EPC] = rel_emb[sl]

        idx = np.stack([_prep_idx(src_c), _prep_idx(dst_c)], axis=2)
        # [NT, P, 2, IDXF]
        msk = np.stack([_prep_msk(src_c), _prep_msk(dst_c)], axis=2)
        # [NT, P, 2, GR, K]
        # edge j at [p = j % 128, k = j // 128] -> rel[t, p, k]
        rel_t = np.ascontiguousarray(
            rel_c.reshape(NT, K, P, DIM).swapaxes(1, 2)
        )
        in_maps.append(
            {
                "table": table,
                "idx": np.ascontiguousarray(idx),
                "rel": rel_t,
                "msk": msk,
            }
        )
    return in_maps


def run_on_hw(node_emb, rel_emb, src, dst, **spmd_kwargs):
    nc = _build_program()
    in_maps = _shard_inputs(node_emb, rel_emb, src, dst)
    res = run_bass_kernel_spmd(nc, in_maps, list(range(N_CORES)), **spmd_kwargs)
    parts = [
        np.asarray(res.results[c]["score"])
        .transpose(0, 2, 1)
        .reshape(EPAD)[:EPC]
        for c in range(N_CORES)
    ]
    return np.concatenate(parts), res


def kernel(node_emb, rel_emb, src, dst):
    scores, _ = run_on_hw(node_emb, rel_emb, src, dst)
    return scores



# revision 5
# speedup vs baseline: 2.9598x; 1.9954x over previous
"""DistMult edge scoring on 8 Trainium2 NeuronCores.

score[e] = sum_d node_emb[src[e], d] * rel_emb[e, d] * node_emb[dst[e], d]

Strategy (v2): exact 256B-row gathers on 4 SWDGE queues.
  - Edges are sorted globally by src and sharded contiguously, so each
    core's src ids span a ~12.5k-row window of the table. The host ships
    that window as a per-core "slice" input; src gather indices are
    slice-local (< 16384, fits the gather's int16 index format) and fetch
    exact 256B rows - no group amplification, no mask-select.
  - dst ids are random over all 100k nodes. int16 indices address at most
    32768 rows, and descriptors must be 256B-multiples, so the table is
    shipped as [50000, 128] row-pairs and each edge's dst is gathered as
    an exact 64-float row from one of 4 static views (half x parity):
    table2[0|25000:+25000, 0:64|64:128]. Within a tile, edges are dealt
    into 4 fixed-capacity class segments (1152 each) so each class is one
    contiguous gather call; pad slots gather row 0 and are dropped on the
    host (their rel is zeroed).
  - dma_gather descgen runs at ~7.9 ns/idx on the Q7 cpu pair selected by
    queue_num. Queue 0 occupies the GpSimd engine for the whole descgen;
    queues 1-3 return in ~0.5us and generate in the background. Tiles are
    issued in pairs with calls spread over all 4 queues/pairs; each
    (parity, dir) semaphore stream stays on one queue so completions are
    FIFO per semaphore.
  - DVE per tile: u = head*rel; u *= tail; reduce -> scores. 3 ops.
"""

import numpy as np

import concourse.bacc as bacc
import concourse.bass as bass
import concourse.mybir as mybir
from concourse import library_config
from concourse.bass_utils import run_bass_kernel_spmd

N_NODES = 100000
DIM = 64
N_EDGES = 1000000
N_CORES = 8

P = 128
EPC = N_EDGES // N_CORES          # 125000
CAP = 1152                        # slots per dst class per tile (9*128)
KC = CAP // P                     # 9 k-slots per class
NCLS = 4                          # dst classes: (half, parity)
TILE = NCLS * CAP                 # 4608 slots per tile
KP = TILE // P                    # 36
NT = 28                           # tiles per core (28*1152 >= 125000/4 + slack)
SRCC = TILE // 2                  # src idxs per sub-call (2 calls per tile)
SIF = SRCC // 16                  # 144 int16 per partition per src call
DIF = CAP // 16                   # 72 per dst call
SLICE_ROWS = 16384
HALF = 50000                      # dst half split (even, so parity survives)

F32 = mybir.dt.float32
I16 = mybir.dt.int16

_cache = {}


def _build_program():
    if "nc" in _cache:
        return _cache["nc"]

    nc = bacc.Bacc(
        "TRN2",
        target_bir_lowering=False,
        debug=False,
        enable_asserts=False,
        num_devices=N_CORES,
        num_swdge_queues=4,
    )
    slice_h = nc.dram_tensor("slice", [SLICE_ROWS, DIM], F32, kind="ExternalInput")
    table2 = nc.dram_tensor("table2", [2 * HALF // 2, 2 * DIM], F32,
                            kind="ExternalInput")  # [50000, 128] row pairs
    sidx_h = nc.dram_tensor("sidx", [NT, P, 2, SIF], I16, kind="ExternalInput")
    didx_h = nc.dram_tensor("didx", [NT, P, NCLS, DIF], I16, kind="ExternalInput")
    rel_h = nc.dram_tensor("rel", [NT, P, KP, DIM], F32, kind="ExternalInput")
    out_h = nc.dram_tensor("score", [NT, P, KP], F32, kind="ExternalOutput")

    # dst gather sources: (half, parity) -> rows [h*25000:+25000], cols
    # [p*64:+64] of table2; row stride 128 elems = elem_step.
    dviews = [
        table2[h * (HALF // 2):(h + 1) * (HALF // 2), p * DIM:(p + 1) * DIM]
        for h in range(2) for p in range(2)
    ]

    NB = 4      # gather/compute tile slots
    NB_IO = 4   # idx/rel prefetch slots
    NBS = 4     # score output slots
    NOPS = 3    # DVE ops per tile

    with (
        nc.Block() as block,
        nc.sbuf_tensor("srcb", [P, NB, KP, DIM], F32) as srcb,
        nc.sbuf_tensor("dstb", [P, NB, KP, DIM], F32) as dstb,
        nc.sbuf_tensor("relb", [P, NB_IO, KP, DIM], F32) as relb,
        nc.sbuf_tensor("sidxb", [P, NB_IO, 2, SIF], I16) as sidxb,
        nc.sbuf_tensor("didxb", [P, NB_IO, NCLS, DIF], I16) as didxb,
        nc.sbuf_tensor("scob", [P, NBS, KP], F32) as scob,
        nc.semaphore("s_sidx") as s_sidx,
        nc.semaphore("s_didx") as s_didx,
        nc.semaphore("s_rel") as s_rel,
        nc.semaphore("s_s0") as s_s0,   # src gathers, tile t%4 == i; even
        nc.semaphore("s_s1") as s_s1,   # tiles on q0, odd on q3. One sem per
        nc.semaphore("s_s2") as s_s2,   # in-flight slot: slot-reuse gating
        nc.semaphore("s_s3") as s_s3,   # (s_vc) orders increments per sem.
        nc.semaphore("s_d0") as s_d0,
        nc.semaphore("s_d1") as s_d1,
        nc.semaphore("s_d2") as s_d2,
        nc.semaphore("s_d3") as s_d3,
        nc.semaphore("s_vc") as s_vc,
        nc.semaphore("s_out") as s_out,
    ):
        s_s = (s_s0, s_s1, s_s2, s_s3)
        s_d = (s_d0, s_d1, s_d2, s_d3)

        @block.sync
        def _(sp: bass.BassEngine):
            # pure prefetcher; completion counts stay ordered by tile
            for t in range(NT):
                s = t % NB_IO
                if t >= 1:
                    sp.wait_ge(s_sidx, 16 * t)
                    sp.wait_ge(s_didx, 16 * t)
                    sp.wait_ge(s_rel, 16 * t)
                if t >= NB_IO:
                    # idx slots free once tile t-NB_IO's gathers retired
                    tt = t - NB_IO
                    sp.wait_ge(s_s[tt % 4], 32 * (tt // 4 + 1))
                    sp.wait_ge(s_d[tt % 4], 64 * (tt // 4 + 1))
                    # rel slot consumed by DVE of tile t-NB_IO
                    sp.wait_ge(s_vc, NOPS * (tt + 1))
                sp.dma_start(out=sidxb[:, s], in_=sidx_h[t]).then_inc(s_sidx, 16)
                sp.dma_start(out=didxb[:, s], in_=didx_h[t]).then_inc(s_didx, 16)
                sp.dma_start(out=relb[:, s], in_=rel_h[t]).then_inc(s_rel, 16)

        @block.scalar
        def _(sc: bass.BassEngine):
            for t in range(NT):
                sc.wait_ge(s_vc, NOPS * (t + 1))
                if t >= 1:
                    sc.wait_ge(s_out, 16 * t)
                sc.dma_start(out=out_h[t], in_=scob[:, t % NBS]).then_inc(
                    s_out, 16
                )
            sc.wait_ge(s_out, 16 * NT)

        @block.gpsimd
        def _(gp: bass.BassGpSimd):
            gp.load_library(library_config.mlp)

            def dst_calls(t):
                s = t % NB
                q = 1 if t % 2 == 0 else 2
                sem = s_d[t % 4]
                for c in range(NCLS):
                    gp.dma_gather(
                        dstb[:, s, c * KC:(c + 1) * KC],
                        dviews[c],
                        didxb[:, t % NB_IO, c],
                        CAP,
                        CAP,
                        DIM,
                        elem_step=2 * DIM,
                        single_packet=False,
                        queue_num=q,
                    ).then_inc(sem, 16)

            def src_calls(t):
                s = t % NB
                q = 0 if t % 2 == 0 else 3
                sem = s_s[t % 4]
                for i in range(2):
                    gp.dma_gather(
                        srcb[:, s, i * (KP // 2):(i + 1) * (KP // 2)],
                        slice_h[:],
                        sidxb[:, t % NB_IO, i],
                        SRCC,
                        SRCC,
                        DIM,
                        elem_step=DIM,
                        single_packet=False,
                        queue_num=q,
                    ).then_inc(sem, 16)

            # issue in tile pairs: all background-queue calls (q1/q2/q3)
            # first, blocking q0 calls last so pairs 1-3 chew their backlog
            # while q0's descgen holds the engine.
            for t0 in range(0, NT, 2):
                t1 = t0 + 1
                gp.wait_ge(s_sidx, 16 * (t1 + 1))
                gp.wait_ge(s_didx, 16 * (t1 + 1))
                if t1 >= NB:
                    gp.wait_ge(s_vc, NOPS * (t1 - NB + 1))
                dst_calls(t0)      # q1, background
                dst_calls(t1)      # q2, background
                src_calls(t1)      # q3, background
                src_calls(t0)      # q0, blocks engine ~36us

        @block.vector
        def _(v: bass.BassEngine):
            mult = mybir.AluOpType.mult
            add = mybir.AluOpType.add
            for t in range(NT):
                s = t % NB
                v.wait_ge(s_s[t % 4], 32 * (t // 4 + 1))
                v.wait_ge(s_d[t % 4], 64 * (t // 4 + 1))
                v.wait_ge(s_rel, 16 * (t + 1))
                if t >= NBS:
                    v.wait_ge(s_out, 16 * (t - NBS + 1))
                if t >= 1:
                    v.wait_ge(s_vc, NOPS * t)
                i = NOPS * t
                v.tensor_tensor(
                    out=srcb[:, s], in0=srcb[:, s], in1=relb[:, t % NB_IO],
                    op=mult,
                ).then_inc(s_vc, 1)
                v.wait_ge(s_vc, i + 1)
                v.tensor_tensor(
                    out=srcb[:, s], in0=srcb[:, s], in1=dstb[:, s], op=mult
                ).then_inc(s_vc, 1)
                v.wait_ge(s_vc, i + 2)
                v.tensor_reduce(
                    out=scob[:, t % NBS],
                    in_=srcb[:, s],
                    axis=mybir.AxisListType.X,
                    op=add,
                ).then_inc(s_vc, 1)

    nc.compile()
    _cache["nc"] = nc
    return nc


def _wrap16(vals):
    """[..., n] int idx -> wrapped [..., 16, n // 16] replicated to 128
    partitions: idx j sits at [j % 16, j // 16]."""
    n = vals.shape[-1]
    lead = vals.shape[:-1]
    w = vals.reshape(*lead, n // 16, 16)
    w = np.swapaxes(w, -1, -2)  # [..., 16, n//16]
    w = np.broadcast_to(
        w[..., None, :, :], (*lead, 8, 16, n // 16)
    ).reshape(*lead, P, n // 16)
    return np.ascontiguousarray(w.astype(np.int16))


def _shard_inputs(node_emb, rel_emb, src, dst):
    node_emb = np.asarray(node_emb, dtype=np.float32)
    rel_emb = np.asarray(rel_emb, dtype=np.float32)
    src = np.asarray(src, dtype=np.int64)
    dst = np.asarray(dst, dtype=np.int64)

    table2 = np.ascontiguousarray(node_emb.reshape(HALF, 2 * DIM))
    order = np.argsort(src, kind="stable")

    in_maps = []
    slot2edge = []
    for c in range(N_CORES):
        eids = order[c * EPC:(c + 1) * EPC]
        s_c = src[eids]
        d_c = dst[eids]
        lo = int(s_c[0])
        span = int(s_c[-1]) - lo + 1
        assert span <= SLICE_ROWS, f"core {c} src span {span}"
        slc = np.zeros((SLICE_ROWS, DIM), np.float32)
        avail = min(SLICE_ROWS, N_NODES - lo)
        slc[:avail] = node_emb[lo:lo + avail]

        cls = (d_c >= HALF) * 2 + (d_c & 1)
        # class-local dst index
        dloc = np.where(d_c >= HALF, (d_c - HALF) >> 1, d_c >> 1)

        # deal each class into NT fixed-capacity tile segments
        slots = np.full((NT, NCLS, CAP), -1, np.int64)  # edge position in eids
        for k in range(NCLS):
            pos = np.nonzero(cls == k)[0]
            assert len(pos) <= NT * CAP, f"class {k} count {len(pos)}"
            flat = slots[:, k, :].reshape(-1)
            flat[:len(pos)] = pos
            slots[:, k, :] = flat.reshape(NT, CAP)

        valid = slots >= 0
        pos_safe = np.where(valid, slots, 0)

        sidx_v = np.where(valid, s_c[pos_safe] - lo, 0)   # [NT, NCLS, CAP]
        didx_v = np.where(valid, dloc[pos_safe], 0)
        rel_v = np.where(
            valid[..., None], rel_emb[eids[pos_safe]], 0.0
        ).astype(np.float32)                               # [NT, NCLS, CAP, D]

        sidx = _wrap16(sidx_v.reshape(NT, 2, SRCC))        # [NT, 2, P, SIF]
        didx = _wrap16(didx_v)                             # [NT, NCLS, P, DIF]

        # slot j of tile t -> (p=j%128, k=j//128); rel must sit at [p, k]
        rel_t = np.ascontiguousarray(
            rel_v.reshape(NT, KP, P, DIM).swapaxes(1, 2)
        )

        in_maps.append(
            {
                "slice": slc,
                "table2": table2,
                "sidx": np.ascontiguousarray(sidx.swapaxes(1, 2)),
                "didx": np.ascontiguousarray(didx.swapaxes(1, 2)),
                "rel": rel_t,
            }
        )
        slot2edge.append(np.where(valid, eids[pos_safe], -1))
    return in_maps, slot2edge


def _unshard(results, slot2edge):
    scores = np.empty(N_EDGES, np.float32)
    for c in range(N_CORES):
        flat = (
            np.asarray(results[c]["score"]).transpose(0, 2, 1).reshape(-1)
        )  # slot j = k*128+p order
        s2e = slot2edge[c].reshape(-1)
        m = s2e >= 0
        scores[s2e[m]] = flat[m]
    return scores


def run_on_hw(node_emb, rel_emb, src, dst, **spmd_kwargs):
    nc = _build_program()
    in_maps, slot2edge = _shard_inputs(node_emb, rel_emb, src, dst)
    res = run_bass_kernel_spmd(nc, in_maps, list(range(N_CORES)), **spmd_kwargs)
    return _unshard(res.results, slot2edge), res


def kernel(node_emb, rel_emb, src, dst):
    scores, _ = run_on_hw(node_emb, rel_emb, src, dst)
    return scores


# revision 6
# speedup vs baseline: 3.6585x; 1.2360x over previous
"""DistMult edge scoring on 8 Trainium2 NeuronCores.

score[e] = sum_d node_emb[src[e], d] * rel_emb[e, d] * node_emb[dst[e], d]

Strategy (v2): exact 256B-row gathers on 4 SWDGE queues.
  - Edges are sorted globally by src and sharded contiguously, so each
    core's src ids span a ~12.5k-row window of the table. The host ships
    that window as a per-core "slice" input; src gather indices are
    slice-local (< 16384, fits the gather's int16 index format) and fetch
    exact 256B rows - no group amplification, no mask-select.
  - dst ids are random over all 100k nodes. int16 indices address at most
    32768 rows, and descriptors must be 256B-multiples, so the table is
    shipped as [50000, 128] row-pairs and each edge's dst is gathered as
    an exact 64-float row from one of 4 static views (half x parity):
    table2[0|25000:+25000, 0:64|64:128]. Within a tile, edges are dealt
    into 4 fixed-capacity class segments (1152 each) so each class is one
    contiguous gather call; pad slots gather row 0 and are dropped on the
    host (their rel is zeroed).
  - dma_gather descgen runs at ~7.9 ns/idx on the Q7 cpu pair selected by
    queue_num. Queue 0 occupies the GpSimd engine for the whole descgen;
    queues 1-3 return in ~0.5us and generate in the background. Tiles are
    issued in pairs with calls spread over all 4 queues/pairs; each
    (parity, dir) semaphore stream stays on one queue so completions are
    FIFO per semaphore.
  - DVE per tile: u = head*rel; u *= tail; reduce -> scores. 3 ops.
"""

import numpy as np

import concourse.bacc as bacc
import concourse.bass as bass
import concourse.mybir as mybir
from concourse import library_config
from concourse.bass_utils import run_bass_kernel_spmd

N_NODES = 100000
DIM = 64
N_EDGES = 1000000
N_CORES = 8

P = 128
EPC = N_EDGES // N_CORES          # 125000
CAP = 1152                        # slots per dst class per tile (9*128)
KC = CAP // P                     # 9 k-slots per class
NCLS = 4                          # dst classes: (half, parity)
TILE = NCLS * CAP                 # 4608 slots per tile
KP = TILE // P                    # 36
NT = 28                           # tiles per core (28*1152 >= 125000/4 + slack)
SIF = CAP // 16                   # 72 int16 per partition per src call
DIF = CAP // 16                   # 72 per dst call
SLICE_ROWS = 16384
HALF = 50000                      # dst half split (even, so parity survives)

F32 = mybir.dt.float32
I16 = mybir.dt.int16

_cache = {}


def _build_program():
    if "nc" in _cache:
        return _cache["nc"]

    nc = bacc.Bacc(
        "TRN2",
        target_bir_lowering=False,
        debug=False,
        enable_asserts=False,
        num_devices=N_CORES,
        num_swdge_queues=4,
    )
    slice_h = nc.dram_tensor("slice", [SLICE_ROWS, DIM], F32, kind="ExternalInput")
    table2 = nc.dram_tensor("table2", [2 * HALF // 2, 2 * DIM], F32,
                            kind="ExternalInput")  # [50000, 128] row pairs
    sidx_h = nc.dram_tensor("sidx", [NT, P, NCLS, SIF], I16, kind="ExternalInput")
    didx_h = nc.dram_tensor("didx", [NT, P, NCLS, DIF], I16, kind="ExternalInput")
    rel_h = nc.dram_tensor("rel", [NT, P, KP, DIM], F32, kind="ExternalInput")
    out_h = nc.dram_tensor("score", [NT, P, KP], F32, kind="ExternalOutput")

    # dst gather sources: (half, parity) -> rows [h*25000:+25000], cols
    # [p*64:+64] of table2; row stride 128 elems = elem_step.
    dviews = [
        table2[h * (HALF // 2):(h + 1) * (HALF // 2), p * DIM:(p + 1) * DIM]
        for h in range(2) for p in range(2)
    ]

    NB = 4      # gather/compute tile slots
    NB_IO = 4   # idx/rel prefetch slots
    NBS = 4     # score output slots
    NOPS = 3    # DVE ops per tile

    with (
        nc.Block() as block,
        nc.sbuf_tensor("srcb", [P, NB, KP, DIM], F32) as srcb,
        nc.sbuf_tensor("dstb", [P, NB, KP, DIM], F32) as dstb,
        nc.sbuf_tensor("relb", [P, NB_IO, KP, DIM], F32) as relb,
        nc.sbuf_tensor("sidxb", [P, NB_IO, NCLS, SIF], I16) as sidxb,
        nc.sbuf_tensor("didxb", [P, NB_IO, NCLS, DIF], I16) as didxb,
        nc.sbuf_tensor("scob", [P, NBS, KP], F32) as scob,
        nc.semaphore("s_sidx") as s_sidx,
        nc.semaphore("s_didx") as s_didx,
        nc.semaphore("s_rel") as s_rel,
        nc.semaphore("s_s0") as s_s0,   # src gathers, tile t%4 == i; even
        nc.semaphore("s_s1") as s_s1,   # tiles on q0, odd on q3. One sem per
        nc.semaphore("s_s2") as s_s2,   # in-flight slot: slot-reuse gating
        nc.semaphore("s_s3") as s_s3,   # (s_vc) orders increments per sem.
        nc.semaphore("s_d0") as s_d0,
        nc.semaphore("s_d1") as s_d1,
        nc.semaphore("s_d2") as s_d2,
        nc.semaphore("s_d3") as s_d3,
        nc.semaphore("s_vc") as s_vc,
        nc.semaphore("s_out") as s_out,
    ):
        s_s = (s_s0, s_s1, s_s2, s_s3)
        s_d = (s_d0, s_d1, s_d2, s_d3)

        @block.sync
        def _(sp: bass.BassEngine):
            # pure prefetcher; completion counts stay ordered by tile
            for t in range(NT):
                s = t % NB_IO
                if t >= 1:
                    sp.wait_ge(s_sidx, 16 * t)
                    sp.wait_ge(s_didx, 16 * t)
                    sp.wait_ge(s_rel, 16 * t)
                if t >= NB_IO:
                    # idx slots free once tile t-NB_IO's gathers retired
                    tt = t - NB_IO
                    sp.wait_ge(s_s[tt % 4], 64 * (tt // 4 + 1))
                    sp.wait_ge(s_d[tt % 4], 64 * (tt // 4 + 1))
                    # rel slot consumed by DVE of tile t-NB_IO
                    sp.wait_ge(s_vc, NOPS * (tt + 1))
                sp.dma_start(out=sidxb[:, s], in_=sidx_h[t]).then_inc(s_sidx, 16)
                sp.dma_start(out=didxb[:, s], in_=didx_h[t]).then_inc(s_didx, 16)
                sp.dma_start(out=relb[:, s], in_=rel_h[t]).then_inc(s_rel, 16)

        @block.scalar
        def _(sc: bass.BassEngine):
            for t in range(NT):
                sc.wait_ge(s_vc, NOPS * (t + 1))
                if t >= 1:
                    sc.wait_ge(s_out, 16 * t)
                sc.dma_start(out=out_h[t], in_=scob[:, t % NBS]).then_inc(
                    s_out, 16
                )
            sc.wait_ge(s_out, 16 * NT)

        @block.gpsimd
        def _(gp: bass.BassGpSimd):
            gp.load_library(library_config.mlp)

            def dst_call(t, c):
                s = t % NB
                gp.dma_gather(
                    dstb[:, s, c * KC:(c + 1) * KC],
                    dviews[c],
                    didxb[:, t % NB_IO, c],
                    CAP,
                    CAP,
                    DIM,
                    elem_step=2 * DIM,
                    single_packet=False,
                    queue_num=1 if t % 2 == 0 else 2,
                ).then_inc(s_d[t % 4], 16)

            def src_call(t, c):
                s = t % NB
                gp.dma_gather(
                    srcb[:, s, c * KC:(c + 1) * KC],
                    slice_h[:],
                    sidxb[:, t % NB_IO, c],
                    CAP,
                    CAP,
                    DIM,
                    elem_step=DIM,
                    single_packet=False,
                    queue_num=0 if t % 2 == 0 else 3,
                ).then_inc(s_s[t % 4], 16)

            # issue in tile pairs, strict queue round-robin q1,q2,q3,q0.
            # A call to a busy Q7 pair stalls the engine until that pair
            # frees, so equal-size calls rotating over all 4 pairs keep
            # every pair fed; effective rate = 4 pairs in parallel.
            for t0 in range(0, NT, 2):
                t1 = t0 + 1
                gp.wait_ge(s_sidx, 16 * (t1 + 1))
                gp.wait_ge(s_didx, 16 * (t1 + 1))
                if t1 >= NB:
                    gp.wait_ge(s_vc, NOPS * (t1 - NB + 1))
                for c in range(NCLS):
                    dst_call(t0, c)   # q1
                    dst_call(t1, c)   # q2
                    src_call(t1, c)   # q3
                    src_call(t0, c)   # q0

        @block.vector
        def _(v: bass.BassEngine):
            mult = mybir.AluOpType.mult
            add = mybir.AluOpType.add
            for t in range(NT):
                s = t % NB
                v.wait_ge(s_s[t % 4], 64 * (t // 4 + 1))
                v.wait_ge(s_d[t % 4], 64 * (t // 4 + 1))
                v.wait_ge(s_rel, 16 * (t + 1))
                if t >= NBS:
                    v.wait_ge(s_out, 16 * (t - NBS + 1))
                if t >= 1:
                    v.wait_ge(s_vc, NOPS * t)
                i = NOPS * t
                v.tensor_tensor(
                    out=srcb[:, s], in0=srcb[:, s], in1=relb[:, t % NB_IO],
                    op=mult,
                ).then_inc(s_vc, 1)
                v.wait_ge(s_vc, i + 1)
                v.tensor_tensor(
                    out=srcb[:, s], in0=srcb[:, s], in1=dstb[:, s], op=mult
                ).then_inc(s_vc, 1)
                v.wait_ge(s_vc, i + 2)
                v.tensor_reduce(
                    out=scob[:, t % NBS],
                    in_=srcb[:, s],
                    axis=mybir.AxisListType.X,
                    op=add,
                ).then_inc(s_vc, 1)

    nc.compile()
    _cache["nc"] = nc
    return nc


def _wrap16(vals):
    """[..., n] int idx -> wrapped [..., 16, n // 16] replicated to 128
    partitions: idx j sits at [j % 16, j // 16]."""
    n = vals.shape[-1]
    lead = vals.shape[:-1]
    w = vals.reshape(*lead, n // 16, 16)
    w = np.swapaxes(w, -1, -2)  # [..., 16, n//16]
    w = np.broadcast_to(
        w[..., None, :, :], (*lead, 8, 16, n // 16)
    ).reshape(*lead, P, n // 16)
    return np.ascontiguousarray(w.astype(np.int16))


def _shard_inputs(node_emb, rel_emb, src, dst):
    node_emb = np.asarray(node_emb, dtype=np.float32)
    rel_emb = np.asarray(rel_emb, dtype=np.float32)
    src = np.asarray(src, dtype=np.int64)
    dst = np.asarray(dst, dtype=np.int64)

    table2 = np.ascontiguousarray(node_emb.reshape(HALF, 2 * DIM))
    order = np.argsort(src, kind="stable")

    in_maps = []
    slot2edge = []
    for c in range(N_CORES):
        eids = order[c * EPC:(c + 1) * EPC]
        s_c = src[eids]
        d_c = dst[eids]
        lo = int(s_c[0])
        span = int(s_c[-1]) - lo + 1
        assert span <= SLICE_ROWS, f"core {c} src span {span}"
        slc = np.zeros((SLICE_ROWS, DIM), np.float32)
        avail = min(SLICE_ROWS, N_NODES - lo)
        slc[:avail] = node_emb[lo:lo + avail]

        cls = (d_c >= HALF) * 2 + (d_c & 1)
        # class-local dst index
        dloc = np.where(d_c >= HALF, (d_c - HALF) >> 1, d_c >> 1)

        # deal each class into NT fixed-capacity tile segments
        slots = np.full((NT, NCLS, CAP), -1, np.int64)  # edge position in eids
        for k in range(NCLS):
            pos = np.nonzero(cls == k)[0]
            assert len(pos) <= NT * CAP, f"class {k} count {len(pos)}"
            flat = slots[:, k, :].reshape(-1)
            flat[:len(pos)] = pos
            slots[:, k, :] = flat.reshape(NT, CAP)

        valid = slots >= 0
        pos_safe = np.where(valid, slots, 0)

        sidx_v = np.where(valid, s_c[pos_safe] - lo, 0)   # [NT, NCLS, CAP]
        didx_v = np.where(valid, dloc[pos_safe], 0)
        rel_v = np.where(
            valid[..., None], rel_emb[eids[pos_safe]], 0.0
        ).astype(np.float32)                               # [NT, NCLS, CAP, D]

        sidx = _wrap16(sidx_v)                             # [NT, NCLS, P, SIF]
        didx = _wrap16(didx_v)                             # [NT, NCLS, P, DIF]

        # slot j of tile t -> (p=j%128, k=j//128); rel must sit at [p, k]
        rel_t = np.ascontiguousarray(
            rel_v.reshape(NT, KP, P, DIM).swapaxes(1, 2)
        )

        in_maps.append(
            {
                "slice": slc,
                "table2": table2,
                "sidx": np.ascontiguousarray(sidx.swapaxes(1, 2)),
                "didx": np.ascontiguousarray(didx.swapaxes(1, 2)),
                "rel": rel_t,
            }
        )
        slot2edge.append(np.where(valid, eids[pos_safe], -1))
    return in_maps, slot2edge


def _unshard(results, slot2edge):
    scores = np.empty(N_EDGES, np.float32)
    for c in range(N_CORES):
        flat = (
            np.asarray(results[c]["score"]).transpose(0, 2, 1).reshape(-1)
        )  # slot j = k*128+p order
        s2e = slot2edge[c].reshape(-1)
        m = s2e >= 0
        scores[s2e[m]] = flat[m]
    return scores


def run_on_hw(node_emb, rel_emb, src, dst, **spmd_kwargs):
    nc = _build_program()
    in_maps, slot2edge = _shard_inputs(node_emb, rel_emb, src, dst)
    res = run_bass_kernel_spmd(nc, in_maps, list(range(N_CORES)), **spmd_kwargs)
    return _unshard(res.results, slot2edge), res


def kernel(node_emb, rel_emb, src, dst):
    scores, _ = run_on_hw(node_emb, rel_emb, src, dst)
    return scores


# revision 8
# speedup vs baseline: 3.8357x; 1.0484x over previous
"""DistMult edge scoring on 8 Trainium2 NeuronCores.

score[e] = sum_d node_emb[src[e], d] * rel_emb[e, d] * node_emb[dst[e], d]

Strategy (v2): exact 256B-row gathers on 4 SWDGE queues.
  - Edges are sorted globally by src and sharded contiguously, so each
    core's src ids span a ~12.5k-row window of the table. The host ships
    that window as a per-core "slice" input; src gather indices are
    slice-local (< 16384, fits the gather's int16 index format) and fetch
    exact 256B rows - no group amplification, no mask-select.
  - dst ids are random over all 100k nodes. int16 indices address at most
    32768 rows, and descriptors must be 256B-multiples, so the table is
    shipped as [50000, 128] row-pairs and each edge's dst is gathered as
    an exact 64-float row from one of 4 static views (half x parity):
    table2[0|25000:+25000, 0:64|64:128]. Within a tile, edges are dealt
    into 4 fixed-capacity class segments (1152 each) so each class is one
    contiguous gather call; pad slots gather row 0 and are dropped on the
    host (their rel is zeroed).
  - dma_gather descgen runs at ~7.9 ns/idx on the Q7 cpu pair selected by
    queue_num. Queue 0 occupies the GpSimd engine for the whole descgen;
    queues 1-3 return in ~0.5us and generate in the background. Tiles are
    issued in pairs with calls spread over all 4 queues/pairs; each
    (parity, dir) semaphore stream stays on one queue so completions are
    FIFO per semaphore.
  - DVE per tile: u = head*rel; u *= tail; reduce -> scores. 3 ops.
"""

from contextlib import ExitStack

import numpy as np

import concourse.bacc as bacc
import concourse.bass as bass
import concourse.mybir as mybir
from concourse import library_config
from concourse.bass_utils import run_bass_kernel_spmd

N_NODES = 100000
DIM = 64
N_EDGES = 1000000
N_CORES = 8

P = 128
EPC = N_EDGES // N_CORES          # 125000
CAP = 1152                        # slots per dst class per tile (9*128)
KC = CAP // P                     # 9 k-slots per class
NCLS = 4                          # dst classes: (half, parity)
TILE = NCLS * CAP                 # 4608 slots per tile
KP = TILE // P                    # 36
NT = 28                           # tiles per core (28*1152 >= 125000/4 + slack)
SIF = CAP // 16                   # 72 int16 per partition per src call
DIF = CAP // 16                   # 72 per dst call
SLICE_ROWS = 16384
HALF = 50000                      # dst half split (even, so parity survives)

F32 = mybir.dt.float32
I16 = mybir.dt.int16

_cache = {}


def _build_program():
    if "nc" in _cache:
        return _cache["nc"]

    nc = bacc.Bacc(
        "TRN2",
        target_bir_lowering=False,
        debug=False,
        enable_asserts=False,
        num_devices=N_CORES,
        num_swdge_queues=4,
    )
    slice_h = nc.dram_tensor("slice", [SLICE_ROWS, DIM], F32, kind="ExternalInput")
    table2 = nc.dram_tensor("table2", [2 * HALF // 2, 2 * DIM], F32,
                            kind="ExternalInput")  # [50000, 128] row pairs
    sidx_h = nc.dram_tensor("sidx", [NT, P, NCLS, SIF], I16, kind="ExternalInput")
    didx_h = nc.dram_tensor("didx", [NT, P, NCLS, DIF], I16, kind="ExternalInput")
    rel_h = nc.dram_tensor("rel", [NT, P, KP, DIM], F32, kind="ExternalInput")
    out_h = nc.dram_tensor("score", [NT, P, KP], F32, kind="ExternalOutput")

    # dst gather sources: (half, parity) -> rows [h*25000:+25000], cols
    # [p*64:+64] of table2; row stride 128 elems = elem_step.
    dviews = [
        table2[h * (HALF // 2):(h + 1) * (HALF // 2), p * DIM:(p + 1) * DIM]
        for h in range(2) for p in range(2)
    ]

    NB = 6      # gather/compute tile slots
    NB_IO = 6   # idx/rel prefetch slots
    NBS = 6     # score output slots
    NOPS = 3    # DVE ops per tile

    with ExitStack() as stack:
        block = stack.enter_context(nc.Block())
        srcb = stack.enter_context(nc.sbuf_tensor("srcb", [P, NB, KP, DIM], F32))
        dstb = stack.enter_context(nc.sbuf_tensor("dstb", [P, NB, KP, DIM], F32))
        relb = stack.enter_context(nc.sbuf_tensor("relb", [P, NB_IO, KP, DIM], F32))
        sidxb = stack.enter_context(
            nc.sbuf_tensor("sidxb", [P, NB_IO, NCLS, SIF], I16)
        )
        didxb = stack.enter_context(
            nc.sbuf_tensor("didxb", [P, NB_IO, NCLS, DIF], I16)
        )
        scob = stack.enter_context(nc.sbuf_tensor("scob", [P, NBS, KP], F32))
        sem = lambda n: stack.enter_context(nc.semaphore(n))
        s_sidx, s_didx, s_rel = sem("s_sidx"), sem("s_didx"), sem("s_rel")
        # one gather sem per in-flight slot per direction: slot-reuse
        # gating (s_vc) orders increments within each sem. even tiles on
        # q0/q1, odd on q2/q3.
        s_s = tuple(sem(f"s_s{i}") for i in range(NB))
        s_d = tuple(sem(f"s_d{i}") for i in range(NB))
        s_vc, s_out = sem("s_vc"), sem("s_out")

        @block.sync
        def _(sp: bass.BassEngine):
            # pure prefetcher; completion counts stay ordered by tile
            for t in range(NT):
                s = t % NB_IO
                if t >= 1:
                    sp.wait_ge(s_sidx, 16 * t)
                    sp.wait_ge(s_didx, 16 * t)
                    sp.wait_ge(s_rel, 16 * t)
                if t >= NB_IO:
                    # idx slots free once tile t-NB_IO's gathers retired
                    tt = t - NB_IO
                    sp.wait_ge(s_s[tt % NB], 64 * (tt // NB + 1))
                    sp.wait_ge(s_d[tt % NB], 64 * (tt // NB + 1))
                    # rel slot consumed by DVE of tile t-NB_IO
                    sp.wait_ge(s_vc, NOPS * (tt + 1))
                sp.dma_start(out=sidxb[:, s], in_=sidx_h[t]).then_inc(s_sidx, 16)
                sp.dma_start(out=didxb[:, s], in_=didx_h[t]).then_inc(s_didx, 16)
                sp.dma_start(out=relb[:, s], in_=rel_h[t]).then_inc(s_rel, 16)

        @block.scalar
        def _(sc: bass.BassEngine):
            for t in range(NT):
                sc.wait_ge(s_vc, NOPS * (t + 1))
                if t >= 1:
                    sc.wait_ge(s_out, 16 * t)
                sc.dma_start(out=out_h[t], in_=scob[:, t % NBS]).then_inc(
                    s_out, 16
                )
            sc.wait_ge(s_out, 16 * NT)

        @block.gpsimd
        def _(gp: bass.BassGpSimd):
            gp.load_library(library_config.mlp)

            def dst_call(t, c):
                s = t % NB
                gp.dma_gather(
                    dstb[:, s, c * KC:(c + 1) * KC],
                    dviews[c],
                    didxb[:, t % NB_IO, c],
                    CAP,
                    CAP,
                    DIM,
                    elem_step=2 * DIM,
                    single_packet=False,
                    queue_num=1 if t % 2 == 0 else 2,
                ).then_inc(s_d[t % NB], 16)

            def src_call(t, c):
                s = t % NB
                gp.dma_gather(
                    srcb[:, s, c * KC:(c + 1) * KC],
                    slice_h[:],
                    sidxb[:, t % NB_IO, c],
                    CAP,
                    CAP,
                    DIM,
                    elem_step=DIM,
                    single_packet=False,
                    queue_num=0 if t % 2 == 0 else 3,
                ).then_inc(s_s[t % NB], 16)

            # issue in tile pairs, strict queue round-robin q1,q2,q3,q0.
            # A call to a busy Q7 pair stalls the engine until that pair
            # frees, so equal-size calls rotating over all 4 pairs keep
            # every pair fed; effective rate = 4 pairs in parallel.
            for t0 in range(0, NT, 2):
                t1 = t0 + 1
                gp.wait_ge(s_sidx, 16 * (t1 + 1))
                gp.wait_ge(s_didx, 16 * (t1 + 1))
                if t1 >= NB:
                    gp.wait_ge(s_vc, NOPS * (t1 - NB + 1))
                for c in range(NCLS):
                    dst_call(t0, c)   # q1
                    dst_call(t1, c)   # q2
                    src_call(t1, c)   # q3
                    src_call(t0, c)   # q0

        @block.vector
        def _(v: bass.BassEngine):
            mult = mybir.AluOpType.mult
            add = mybir.AluOpType.add
            for t in range(NT):
                s = t % NB
                v.wait_ge(s_s[t % NB], 64 * (t // NB + 1))
                v.wait_ge(s_d[t % NB], 64 * (t // NB + 1))
                v.wait_ge(s_rel, 16 * (t + 1))
                if t >= NBS:
                    v.wait_ge(s_out, 16 * (t - NBS + 1))
                if t >= 1:
                    v.wait_ge(s_vc, NOPS * t)
                i = NOPS * t
                v.tensor_tensor(
                    out=srcb[:, s], in0=srcb[:, s], in1=relb[:, t % NB_IO],
                    op=mult,
                ).then_inc(s_vc, 1)
                v.wait_ge(s_vc, i + 1)
                v.tensor_tensor(
                    out=srcb[:, s], in0=srcb[:, s], in1=dstb[:, s], op=mult
                ).then_inc(s_vc, 1)
                v.wait_ge(s_vc, i + 2)
                v.tensor_reduce(
                    out=scob[:, t % NBS],
                    in_=srcb[:, s],
                    axis=mybir.AxisListType.X,
                    op=add,
                ).then_inc(s_vc, 1)

    nc.compile()
    _cache["nc"] = nc
    return nc


def _wrap16(vals):
    """[..., n] int idx -> wrapped [..., 16, n // 16] replicated to 128
    partitions: idx j sits at [j % 16, j // 16]."""
    n = vals.shape[-1]
    lead = vals.shape[:-1]
    w = vals.reshape(*lead, n // 16, 16)
    w = np.swapaxes(w, -1, -2)  # [..., 16, n//16]
    w = np.broadcast_to(
        w[..., None, :, :], (*lead, 8, 16, n // 16)
    ).reshape(*lead, P, n // 16)
    return np.ascontiguousarray(w.astype(np.int16))


def _shard_inputs(node_emb, rel_emb, src, dst):
    node_emb = np.asarray(node_emb, dtype=np.float32)
    rel_emb = np.asarray(rel_emb, dtype=np.float32)
    src = np.asarray(src, dtype=np.int64)
    dst = np.asarray(dst, dtype=np.int64)

    table2 = np.ascontiguousarray(node_emb.reshape(HALF, 2 * DIM))
    order = np.argsort(src, kind="stable")

    in_maps = []
    slot2edge = []
    for c in range(N_CORES):
        eids = order[c * EPC:(c + 1) * EPC]
        s_c = src[eids]
        d_c = dst[eids]
        lo = int(s_c[0])
        span = int(s_c[-1]) - lo + 1
        assert span <= SLICE_ROWS, f"core {c} src span {span}"
        slc = np.zeros((SLICE_ROWS, DIM), np.float32)
        avail = min(SLICE_ROWS, N_NODES - lo)
        slc[:avail] = node_emb[lo:lo + avail]

        cls = (d_c >= HALF) * 2 + (d_c & 1)
        # class-local dst index
        dloc = np.where(d_c >= HALF, (d_c - HALF) >> 1, d_c >> 1)

        # deal each class into NT fixed-capacity tile segments
        slots = np.full((NT, NCLS, CAP), -1, np.int64)  # edge position in eids
        for k in range(NCLS):
            pos = np.nonzero(cls == k)[0]
            assert len(pos) <= NT * CAP, f"class {k} count {len(pos)}"
            flat = slots[:, k, :].reshape(-1)
            flat[:len(pos)] = pos
            slots[:, k, :] = flat.reshape(NT, CAP)

        valid = slots >= 0
        pos_safe = np.where(valid, slots, 0)

        sidx_v = np.where(valid, s_c[pos_safe] - lo, 0)   # [NT, NCLS, CAP]
        didx_v = np.where(valid, dloc[pos_safe], 0)
        rel_v = np.where(
            valid[..., None], rel_emb[eids[pos_safe]], 0.0
        ).astype(np.float32)                               # [NT, NCLS, CAP, D]

        sidx = _wrap16(sidx_v)                             # [NT, NCLS, P, SIF]
        didx = _wrap16(didx_v)                             # [NT, NCLS, P, DIF]

        # slot j of tile t -> (p=j%128, k=j//128); rel must sit at [p, k]
        rel_t = np.ascontiguousarray(
            rel_v.reshape(NT, KP, P, DIM).swapaxes(1, 2)
        )

        in_maps.append(
            {
                "slice": slc,
                "table2": table2,
                "sidx": np.ascontiguousarray(sidx.swapaxes(1, 2)),
                "didx": np.ascontiguousarray(didx.swapaxes(1, 2)),
                "rel": rel_t,
            }
        )
        slot2edge.append(np.where(valid, eids[pos_safe], -1))
    return in_maps, slot2edge


def _unshard(results, slot2edge):
    scores = np.empty(N_EDGES, np.float32)
    for c in range(N_CORES):
        flat = (
            np.asarray(results[c]["score"]).transpose(0, 2, 1).reshape(-1)
        )  # slot j = k*128+p order
        s2e = slot2edge[c].reshape(-1)
        m = s2e >= 0
        scores[s2e[m]] = flat[m]
    return scores


def run_on_hw(node_emb, rel_emb, src, dst, **spmd_kwargs):
    nc = _build_program()
    in_maps, slot2edge = _shard_inputs(node_emb, rel_emb, src, dst)
    res = run_bass_kernel_spmd(nc, in_maps, list(range(N_CORES)), **spmd_kwargs)
    return _unshard(res.results, slot2edge), res


def kernel(node_emb, rel_emb, src, dst):
    scores, _ = run_on_hw(node_emb, rel_emb, src, dst)
    return scores
